# revision 16
# baseline (speedup 1.0000x reference)
"""Trainium2 Bass kernel for nn_AttLayer_9972914061697 (sparse_attention).

Reference computation (jax):
    q, k, v = split(x, 3, axis=-1)              # x: [B=4, T=4096, 3C=384]
    score   = einsum('btc,bsc->bts', k, q) / sqrt(C)
    out     = softmax(score, -1) @ v            # [B, T, C=128]

Sharding: 8 cores = 4 batches x 2 T-halves (data parallel, zero comm).
Each core holds full q, v of its batch plus its 2048-row k chunk and
produces its 2048-row output chunk. q and k are shipped host-transposed
([C, T] layout) so no XBAR-transpose DMAs are needed; all tensors bf16.

Per-core algorithm (v2):
  - S_T[s, t] = sum_c q[s,c] k[t,c] via qT_chunk.T @ kT, staged in PSUM as
    4 stages of [128, 512] per s-chunk (2 rotating bank tiles).
  - P_T = exp(S_T / sqrt(C)) split across two engines per chunk:
    stages 0,2 on ScalarE (table exp), stages 1,3 on VectorE via the
    Schraudolph bit-trick: bf16 bits of exp(x) ~= int16(x*A + B), computed
    as one tensor_scalar (mult, add) with int16 output, bitcast to bf16.
  - PV accumulates over ALL 32 s-chunks directly in PSUM: the 16 [128,129]
    output accumulators ([t-tile, v|rowsum]) are packed 3-per-bank at
    130-column stride in 6 bank tiles.  A zeroing matmul (start=True) per
    bank sets every element's has_written bit up front, so the 3
    interleaved accumulation chains per bank all run with start=False.
  - Tail: per bank, VectorE reciprocal of the 3 rowsum columns, then
    per-tile tensor_scalar/scalar-mul (split DVE/ACT) into an SBUF staging
    tile and DMA out.
"""

import numpy as np
import ml_dtypes

import concourse.bass as bass
import concourse.tile as tile
from concourse import bacc, mybir
from concourse.bass_utils import run_bass_kernel_spmd
from concourse.alu_op_type import AluOpType

F32 = mybir.dt.float32
BF16 = mybir.dt.bfloat16
I16 = mybir.dt.int16

B = 4
T = 4096
C = 128
N_CORES = 8
TL = T // 2            # 2048 t-rows per core
NSC = T // 128         # 32 s-chunks
NTT = TL // 128        # 16 t-tiles
STG = 512              # S staging width (one PSUM bank)
NSTG = TL // STG       # 4 stages per s-chunk
VW = 132               # v chunk pitch: 128 v cols + ones col + pad (8B align)

SCALE = 1.0 / float(np.sqrt(C))
LOG2E = float(np.log2(np.e))
SCH_A = float(np.sqrt(128.0)) * LOG2E          # x*A maps raw score to 128*log2(P)
SCH_B = 127.0 * 128.0 - 5.77                   # bias, calibrated for round-to-nearest

# per-pass out accumulators: 8 t-tiles -> 3 banks, 3 tiles per bank at
# 130-col stride (last bank holds 2)
OBANK = [(lt // 3, 130 * (lt % 3)) for lt in range(8)]
NBANK = 3
BANKW = [390, 390, 260]
BTILES = [3, 3, 2]


def build_nc():
    nc = bacc.Bacc()
    qT = nc.declare_dram_parameter("qT", [C, T], BF16, isOutput=False)
    kT = nc.declare_dram_parameter("kT", [C, TL], BF16, isOutput=False)
    v = nc.declare_dram_parameter("v", [T, C], BF16, isOutput=False)
    out = nc.declare_dram_parameter("out", [TL, C], F32, isOutput=True)

    vw = v[:].rearrange("(n p) c -> p n c", p=128)    # [128, 32, 128]
    ov = out[:].rearrange("(n p) c -> p n c", p=128)  # [128, 16, 128]

    with tile.TileContext(nc) as tc:
        with (
            tc.tile_pool(name="const", bufs=1) as const_pool,
            tc.tile_pool(name="qkt", bufs=1) as qkt_pool,
            tc.tile_pool(name="vbuf", bufs=1) as v_pool,
            tc.tile_pool(name="pT", bufs=6) as pT_pool,
            tc.tile_pool(name="ost", bufs=4) as ost_pool,
            tc.tile_pool(name="spsum", bufs=4, space="PSUM") as spsum,
            tc.tile_pool(name="opsum", bufs=1, space="PSUM") as opsum,
        ):
            qT_t = qkt_pool.tile([128, T], BF16, tag="qT")
            kT_t = qkt_pool.tile([128, TL], BF16, tag="kT")
            vv = v_pool.tile([128, NSC * VW], BF16)
            vv3 = vv[:].rearrange("p (n c) -> p n c", c=VW)
            zer = const_pool.tile([128, 390], BF16, tag="zer")
            rcp = const_pool.tile([128, NTT], F32, tag="rcp")

            nc.vector.memset(zer[:], 0.0)
            nc.vector.memset(vv3[:, :, C : C + 1], 1.0)

            # warm the ACT exp table so the ~2.7us table load overlaps
            # the prologue DMA instead of stalling the first real exp
            warm = const_pool.tile([128, 8], F32, tag="warm")
            nc.vector.memset(warm[:], 0.0)
            nc.scalar.activation(
                warm[:], warm[:], mybir.ActivationFunctionType.Exp, scale=1.0
            )

            def load_q(piece):  # 512 qT cols = 4 chunks
                nc.sync.dma_start(
                    out=qT_t[:, piece * 512 : (piece + 1) * 512],
                    in_=qT[:, piece * 512 : (piece + 1) * 512],
                )

            def load_v(piece):  # 4 v chunks
                nc.sync.dma_start(
                    out=vv3[:, piece * 4 : piece * 4 + 4, 0:C],
                    in_=vw[:, piece * 4 : piece * 4 + 4, :],
                )

            # ALL input DMA issued up front on the sync HWDGE queue in
            # urgency order — the queue streams ~2.5MB in ~7us with no
            # semaphore dependencies, far ahead of every consumer
            nc.sync.dma_start(out=kT_t[:, 0:512], in_=kT[:, 0:512])
            load_q(0)
            nc.sync.dma_start(out=kT_t[:, 512:1024], in_=kT[:, 512:1024])
            load_v(0)
            load_q(1)
            load_v(1)
            load_q(2)
            nc.sync.dma_start(out=kT_t[:, 1024:2048], in_=kT[:, 1024:2048])
            for piece in range(2, 8):
                load_v(piece)
                load_q(piece + 1) if piece + 1 < 8 else None

            def qk_stage(ph, j, pT_j, pT_j16, st):
                # st is pass-local (0,1); global t-stage is 2*ph+st
                gst = 2 * ph + st
                lhs = qT_t[:, j * 128 : (j + 1) * 128]
                s_st = spsum.tile([128, STG], F32, tag="S")
                nc.tensor.matmul(
                    s_st[:], lhs, kT_t[:, gst * STG : (gst + 1) * STG],
                    start=True, stop=True,
                )
                dst = slice(st * STG, (st + 1) * STG)
                if st % 2 == 0:
                    nc.scalar.activation(
                        pT_j[:, dst], s_st[:],
                        mybir.ActivationFunctionType.Exp, scale=SCALE,
                    )
                else:
                    nc.vector.tensor_scalar(
                        pT_j16[:, dst], s_st[:], SCH_A, SCH_B,
                        AluOpType.mult, AluOpType.add,
                    )

            def pv_tiles(obank, ph, j, pT_j, final, flush_inline=False):
                for lt in range(8):
                    b_, off = OBANK[lt]
                    nc.tensor.matmul(
                        obank[b_][:, off : off + 129],
                        pT_j[:, lt * 128 : (lt + 1) * 128],
                        vv3[:, j, 0 : C + 1],
                        start=False, stop=final, skip_group_check=True,
                    )
                    if flush_inline and lt + 1 in (3, 6, 8):
                        flush_bank(obank, ph, lt // 3, spread_q=True)

            rcp3 = rcp[:].rearrange("p (t o) -> p t o", o=1)

            def flush_bank(obank, ph, b_, spread_q=False):
                ntile = BTILES[b_]
                bank = obank[b_]
                b3 = bank[:].rearrange("p (t x) -> p t x", x=130)
                t0 = 8 * ph + b_ * 3
                nc.vector.reciprocal(
                    rcp3[:, t0 : t0 + ntile, :], b3[:, 0:ntile, 128:129]
                )
                ost = ost_pool.tile([128, 3 * 128], F32, tag="ost")
                ost3 = ost[:].rearrange("p (t c) -> p t c", c=128)
                for i in range(ntile):
                    tt = t0 + i
                    off = 130 * i
                    if tt % 2 == 0:
                        nc.vector.tensor_scalar_mul(
                            ost[:, i * 128 : (i + 1) * 128],
                            bank[:, off : off + 128],
                            rcp[:, tt : tt + 1],
                        )
                    else:
                        nc.scalar.mul(
                            ost[:, i * 128 : (i + 1) * 128],
                            bank[:, off : off + 128],
                            rcp[:, tt : tt + 1],
                        )
                eng = [nc.sync, nc.scalar, nc.sync][b_] if spread_q else nc.sync
                eng.dma_start(
                    out=ov[:, t0 : t0 + ntile, :], in_=ost3[:, 0:ntile, :]
                )

            # ---- two t-half passes over all 32 s-chunks ----
            # pass ph covers t-cols [ph*1024, (ph+1)*1024) = out tiles 8ph..8ph+7.
            # Only 3 PSUM banks of accumulators per pass, so S staging gets 5
            # rotating banks and the QK->exp->QK WAR chain never binds.
            # Pass A's flush overlaps pass B's compute.
            def zero_mm(bank, width):
                # start=True writes zeros + sets every element's has_written
                # bit so the 3 packed chains per bank all run start=False
                nc.tensor.matmul(
                    bank[:, 0:width], zer[:, 0:128], zer[:, 0:width],
                    start=True, stop=True,
                )

            pending = None  # pass A's (obank, ph) awaiting flush during pass B
            for ph in range(2):
                obank = [
                    opsum.tile([128, BANKW[b_]], F32, tag=f"ob{b_}", name=f"ob{b_}")
                    for b_ in range(NBANK)
                ]
                if pending is None:
                    for b_ in range(NBANK):
                        zero_mm(obank[b_], BANKW[b_])
                # PV runs TWO chunks behind QK/exp so the PV LDWEIGHTS never
                # waits on a just-finished exp
                prevs = []
                for j in range(NSC):
                    pT_j = pT_pool.tile([128, 2 * STG], BF16, tag="pT")
                    pT_j16 = pT_j[:].bitcast(I16)
                    qk_stage(ph, j, pT_j, pT_j16, 0)
                    qk_stage(ph, j, pT_j, pT_j16, 1)
                    prevs.append(pT_j)
                    if pending is not None and j < NBANK:
                        # spread the previous pass's flush + this pass's
                        # zeroing over the first chunks (one bank per chunk)
                        # so the exp engines never see a burst of flush work
                        flush_bank(*pending, j)
                        zero_mm(obank[j], BANKW[j])
                    if j >= 2:
                        pv_tiles(obank, ph, j - 2, prevs[j - 2], final=False)
                pv_tiles(obank, ph, NSC - 2, prevs[NSC - 2], final=False)
                pv_tiles(
                    obank, ph, NSC - 1, prevs[NSC - 1],
                    final=True, flush_inline=(ph == 1),
                )
                pending = (obank, ph)

    nc.finalize()
    return nc


_NC_CACHE = None


def make_in_maps(x: np.ndarray):
    xb = np.asarray(x, dtype=np.float32).astype(ml_dtypes.bfloat16)
    in_maps = []
    for core in range(N_CORES):
        b, th = core // 2, core % 2
        in_maps.append(
            {
                "qT": np.ascontiguousarray(xb[b, :, 0:C].T),
                "kT": np.ascontiguousarray(xb[b, th * TL : (th + 1) * TL, C : 2 * C].T),
                "v": np.ascontiguousarray(xb[b, :, 2 * C : 3 * C]),
            }
        )
    return in_maps


def kernel(x: np.ndarray) -> np.ndarray:
    global _NC_CACHE
    x = np.asarray(x, dtype=np.float32)
    assert x.shape == (B, T, 3 * C), x.shape

    if _NC_CACHE is None:
        _NC_CACHE = build_nc()
    nc = _NC_CACHE

    res = run_bass_kernel_spmd(nc, make_in_maps(x), core_ids=list(range(N_CORES)))

    out = np.empty((B, T, C), dtype=np.float32)
    for core in range(N_CORES):
        b, th = core // 2, core % 2
        out[b, th * TL : (th + 1) * TL] = res.results[core]["out"]
    return out
